# revision 12
# baseline (speedup 1.0000x reference)
"""ComplexPolarAttention Trainium2 kernel (8-core SPMD, row-sharded) — v3.

Math (matching the reference):
  c = mag*cos(phase); s = mag*sin(phase)
  scores = c@c.T + s@s.T + bias     (bias: sparse edge scatter, last-dup-wins)
  attn = softmax(scores, axis=1)
  out = (attn@mag, attn@phase)

v3 design (per core, 1024 query rows x 8192 keys):
  xt [128 feat, 8192 nodes] f16 built via range-wrap + Sin + PE transposes
  (all Sin ACTs ordered before any Exp ACT: one table switch).
  QK: matmul(lhsT=xt_kc [128f,128k], rhs=xtq [128f,512q]) f16, N=512 wide;
      2 kc per [128,1024] fp32 PSUM tile (2 banks), double buffered.
  exp: one ACT per [128,1024] PSUM tile -> P0 bf16 SBUF (no bias yet).
  bias: P2 = (1 + scatter(expm1(es))) * P0 via ts(+1, 4x) + TT(mult, 2x);
      es computed on-device from bucketed edge_attr; gpsimd local_scatter.
  PV: 8 narrow MMs per group, lhsT=P2 chunks [128k,128q] (weights, FWL),
      rhs=mpo [128k,129] = [mag|phase|ones] bf16 -> out [q,129] PSUM accum;
      col 128 is the softmax denominator (measured 57ns/MM spacing).
  epilogue: per 128-q chunk, out = pv[:,0:128] * 1/pv[:,128] -> [NQ,128] DRAM.
"""
import os
import sys

sys.path.insert(0, "/opt/trn_rl_repo")

# The NTFF profile hook module is missing from this image's antenv package;
# bass_utils imports it unconditionally when tracing. Create it if absent so
# BASS_TRACE=1 works (degrades silently if dirs are read-only).
_HOOK_SRC = '''_hook = None

def set_axon_ntff_profile_hook(hook):
    global _hook
    _hook = hook

def get_axon_ntff_profile_hook():
    return _hook
'''
for _d in ("/opt/trn_rl_repo/antenv", "/root/.axon_site/_ro/trn_rl_repo/antenv"):
    try:
        _p = os.path.join(_d, "axon_hooks.py")
        if os.path.isdir(_d) and not os.path.exists(_p):
            with open(_p, "w") as _f:
                _f.write(_HOOK_SRC)
    except OSError:
        pass

import math
import numpy as np

import concourse.bass as bass
import concourse.mybir as mybir
import concourse.tile as tile
from concourse import bacc
from concourse.bass_utils import run_bass_kernel_spmd
from concourse.masks import make_identity

N, D, E, EDGE_DIM = 8192, 64, 262144, 4
CORES = 8
NQ = N // CORES          # 1024 query rows per core
QH = 512                 # query half width (QK rhs stream width)
N_QH = NQ // QH          # 2
KC = 128                 # key chunk width
N_KC = N // KC           # 64 key chunks
KCG = 2                  # key chunks per exp/scatter group
N_G = N_KC // KCG        # 32 groups
GW = KCG * QH            # 1024 group tile width
NTQ = NQ // 128          # 8 query tiles
CHT = 16                 # key tiles per stage-A chunk
N_CH = N_KC // CHT       # 4 chunks
MPW = 132                # padded [mag|phase|ones] stride in mpo

f32 = mybir.dt.float32
f16 = mybir.dt.float16
bf16 = mybir.dt.bfloat16
i16 = mybir.dt.int16
AF = mybir.ActivationFunctionType
ALU = mybir.AluOpType

_cache = {}
LAST_RESULTS = None


def _build(slots):
    tot = N_QH * N_G * slots
    nc = bacc.Bacc("TRN2", target_bir_lowering=False, debug=False,
                   num_devices=CORES)
    phase_d = nc.dram_tensor("phase", (N, D), f32, kind="ExternalInput")
    phq_d = nc.dram_tensor("phq", (NQ, D), f32, kind="ExternalInput")
    mag_d = nc.dram_tensor("mag", (N, D), f16, kind="ExternalInput")
    mq_d = nc.dram_tensor("mq", (NQ, D), f16, kind="ExternalInput")
    mpo_d = nc.dram_tensor("mpo", (N, MPW), bf16, kind="ExternalInput")
    eidx_d = nc.dram_tensor("eidx", (128, tot), i16, kind="ExternalInput")
    eattr_d = nc.dram_tensor("eattr", (128, 4 * tot), f16, kind="ExternalInput")
    w_d = nc.dram_tensor("W", (D, EDGE_DIM), f32, kind="ExternalInput")
    b_d = nc.dram_tensor("bvec", (D, 1), f32, kind="ExternalInput")
    out_d = nc.dram_tensor("out", (NQ, 128), f32, kind="ExternalOutput")

    with tile.TileContext(nc) as tc, \
         tc.tile_pool(name="persist", bufs=1) as pers:
        xt = pers.tile([128, N], f16, tag="xt")
        xtq = pers.tile([128, NQ], f16, tag="xtq")
        mpo = pers.tile([128, N_KC * MPW], bf16, tag="mpo")
        magb = pers.tile([128, N_KC * D], f16, tag="magb")
        esb_m = pers.tile([128, tot], f16, tag="esb_m")
        eidx_sb = pers.tile([128, tot], i16, tag="eidx_sb")
        ident = pers.tile([128, 128], f16, tag="ident")
        ones1 = pers.tile([1, 128], f32, tag="ones1")

        mpo3 = mpo[:].rearrange("p (t f) -> p t f", f=MPW)
        mpo_r = mpo_d[:].rearrange("(t p) f -> p t f", p=128)
        magb3 = magb[:].rearrange("p (t d) -> p t d", d=D)
        mag_r = mag_d[:].rearrange("(t p) d -> p t d", p=128)

        # ---- W/b prep: wbc[128,4] = broadcast W.sum(0), bbc[128,1] = b.sum()
        wbc = pers.tile([128, EDGE_DIM], f32, tag="wbc")
        bbc = pers.tile([128, 1], f32, tag="bbc")
        with tc.tile_pool(name="eprep", bufs=1) as ep, \
             tc.tile_pool(name="eprep_ps", bufs=1, space="PSUM") as epp:
            w_sb = ep.tile([D, EDGE_DIM], f32, tag="w_sb")
            nc.sync.dma_start(out=w_sb[:], in_=w_d[:])
            b_sb = ep.tile([D, 1], f32, tag="b_sb")
            nc.sync.dma_start(out=b_sb[:], in_=b_d[:])
            ones64 = ep.tile([D, 1], f32, tag="ones64")
            nc.vector.memset(ones64[:], 1.0)
            make_identity(nc, ident[:])
            nc.vector.memset(ones1[:], 1.0)

            ws_ps = epp.tile([1, EDGE_DIM], f32, tag="ws_ps")
            nc.tensor.matmul(out=ws_ps[:], lhsT=ones64[:], rhs=w_sb[:],
                             start=True, stop=True)
            ws_row = ep.tile([1, EDGE_DIM], f32, tag="ws_row")
            nc.scalar.copy(out=ws_row[:], in_=ws_ps[:])
            bs_ps = epp.tile([1, 1], f32, tag="bs_ps")
            nc.tensor.matmul(out=bs_ps[:], lhsT=b_sb[:], rhs=ones64[:],
                             start=True, stop=True)
            bs_row = ep.tile([1, 1], f32, tag="bs_row")
            nc.scalar.copy(out=bs_row[:], in_=bs_ps[:])
            wbc_ps = epp.tile([128, EDGE_DIM], f32, tag="wbc_ps")
            nc.tensor.matmul(out=wbc_ps[:], lhsT=ones1[:], rhs=ws_row[:],
                             start=True, stop=True)
            nc.scalar.copy(out=wbc[:], in_=wbc_ps[:])
            bbc_ps = epp.tile([128, 1], f32, tag="bbc_ps")
            nc.tensor.matmul(out=bbc_ps[:], lhsT=ones1[:], rhs=bs_row[:],
                             start=True, stop=True)
            nc.scalar.copy(out=bbc[:], in_=bbc_ps[:])

        with tc.tile_pool(name="a_ph", bufs=5) as php, \
             tc.tile_pool(name="a_ws", bufs=2) as wsp, \
             tc.tile_pool(name="a_trig", bufs=4) as trp, \
             tc.tile_pool(name="a_cs", bufs=4) as csp, \
             tc.tile_pool(name="a_ps", bufs=1, space="PSUM") as apsp, \
             tc.tile_pool(name="edges", bufs=1) as egp, \
             tc.tile_pool(name="b_qk", bufs=2, space="PSUM") as qkp, \
             tc.tile_pool(name="b_pv", bufs=1, space="PSUM") as pvp, \
             tc.tile_pool(name="b_bias", bufs=3) as bp, \
             tc.tile_pool(name="b_p2", bufs=3) as p2p, \
             tc.tile_pool(name="b_out", bufs=4) as obp:

            pha_r = phase_d[:].rearrange("(t p) d -> p t d", p=128)
            phq_r = phq_d[:].rearrange("(t p) d -> p t d", p=128)
            mq_r = mq_d[:].rearrange("(t p) d -> p t d", p=128)

            # query side trig (Sin ACTs first in the scalar queue)
            phqb = php.tile([128, NTQ * D], f32, tag="phqb")
            phqb3 = phqb[:].rearrange("p (t d) -> p t d", d=D)
            nc.sync.dma_start(out=phqb3[:], in_=phq_r[:])
            mqb = egp.tile([128, NTQ * D], f16, tag="mqb")
            mqb3 = mqb[:].rearrange("p (t d) -> p t d", d=D)
            nc.sync.dma_start(out=mqb3[:], in_=mq_r[:])
            wsq = wsp.tile([128, NTQ * 128], f32, tag="wsq")
            wsq3 = wsq[:].rearrange("p (t x) -> p t x", x=128)
            nc.vector.add_range_wrap(out=wsq3[:, :, 0:D], in_=phqb3[:],
                                     shift=math.pi / 2, bound=math.pi,
                                     period=2 * math.pi)
            nc.vector.add_range_wrap(out=wsq3[:, :, D:128], in_=phqb3[:],
                                     shift=0.0, bound=math.pi,
                                     period=2 * math.pi)
            trigq = trp.tile([128, NTQ * 128], f32, tag="trigq")
            nc.scalar.activation(out=trigq[:], in_=wsq[:], func=AF.Sin)

            # stage A key chunks: DMA + wrap + Sin up front, chunked DMAs in
            # dependency order (phase/mag gate trig; mpo gates PV; edge
            # tensors gate the first scatter)
            trig_ch = []
            ph_tiles = []
            for h in range(N_CH):
                phb = php.tile([128, CHT * D], f32, tag="phb",
                               name=f"phb{h}")
                phb3 = phb[:].rearrange("p (t d) -> p t d", d=D)
                nc.sync.dma_start(out=phb3[:],
                                  in_=pha_r[:, h * CHT:(h + 1) * CHT, :])
                nc.sync.dma_start(out=magb3[:, h * CHT:(h + 1) * CHT, :],
                                  in_=mag_r[:, h * CHT:(h + 1) * CHT, :])
                if h == 0:
                    nc.sync.dma_start(out=mpo3[:, 0:CHT, :],
                                      in_=mpo_r[:, 0:CHT, :])
                    nc.sync.dma_start(out=eidx_sb[:], in_=eidx_d[:])
                ph_tiles.append((phb, phb3))
            for h in range(N_CH):
                phb, phb3 = ph_tiles[h]
                ws = wsp.tile([128, CHT * 128], f32, tag="ws")
                ws3 = ws[:].rearrange("p (t x) -> p t x", x=128)
                nc.vector.add_range_wrap(out=ws3[:, :, 0:D], in_=phb3[:],
                                         shift=math.pi / 2, bound=math.pi,
                                         period=2 * math.pi)
                nc.vector.add_range_wrap(out=ws3[:, :, D:128], in_=phb3[:],
                                         shift=0.0, bound=math.pi,
                                         period=2 * math.pi)
                trig = trp.tile([128, CHT * 128], f32, tag="trig")
                nc.scalar.activation(out=trig[:], in_=ws[:], func=AF.Sin)
                trig_ch.append(trig)

            # query side cs + transposes -> xtq (PSUM->SBUF copies on scalar)
            csq = csp.tile([128, NTQ * 128], f16, tag="csq")
            csq3 = csq[:].rearrange("p (t x) -> p t x", x=128)
            trigq3 = trigq[:].rearrange("p (t x) -> p t x", x=128)
            nc.vector.tensor_tensor(out=csq3[:, :, 0:D], in0=trigq3[:, :, 0:D],
                                    in1=mqb3[:], op=ALU.mult)
            nc.vector.tensor_tensor(out=csq3[:, :, D:128],
                                    in0=trigq3[:, :, D:128],
                                    in1=mqb3[:], op=ALU.mult)
            for qd in range(NTQ // 4):
                tp = apsp.tile([128, 512], f16, tag="tp")
                for k in range(4):
                    i = qd * 4 + k
                    nc.tensor.transpose(out=tp[:, k * 128:(k + 1) * 128],
                                        in_=csq[:, i * 128:(i + 1) * 128],
                                        identity=ident[:])
                nc.vector.tensor_copy(out=xtq[:, qd * 512:(qd + 1) * 512],
                                      in_=tp[:])

            # ---- edge scores (eidx DMA already issued with chunk 0)
            ea = egp.tile([128, 4 * tot], f16, tag="ea")
            nc.sync.dma_start(out=ea[:], in_=eattr_d[:])
            for h in range(1, N_CH):
                nc.sync.dma_start(out=mpo3[:, h * CHT:(h + 1) * CHT, :],
                                  in_=mpo_r[:, h * CHT:(h + 1) * CHT, :])
            acc_a = egp.tile([128, tot], f32, tag="acc_a")
            acc_b = egp.tile([128, tot], f32, tag="acc_b")
            nc.vector.tensor_scalar(acc_a[:], ea[:, 0:tot], wbc[:, 0:1], None,
                                    ALU.mult)
            nc.vector.scalar_tensor_tensor(out=acc_b[:], in0=ea[:, tot:2 * tot],
                                           scalar=wbc[:, 1:2], in1=acc_a[:],
                                           op0=ALU.mult, op1=ALU.add)
            nc.vector.scalar_tensor_tensor(out=acc_a[:], in0=ea[:, 2 * tot:3 * tot],
                                           scalar=wbc[:, 2:3], in1=acc_b[:],
                                           op0=ALU.mult, op1=ALU.add)
            nc.vector.scalar_tensor_tensor(out=acc_b[:], in0=ea[:, 3 * tot:4 * tot],
                                           scalar=wbc[:, 3:4], in1=acc_a[:],
                                           op0=ALU.mult, op1=ALU.add)
            nc.vector.tensor_scalar(esb_m[:], acc_b[:], bbc[:, 0:1], None,
                                    ALU.add)

            # ---- main loop
            def emit_chunk(h):
                """cs-mult + transposes + scalar copies for key chunk h."""
                trig3 = trig_ch[h][:].rearrange("p (t x) -> p t x", x=128)
                cs = csp.tile([128, CHT * 128], f16, tag="cs")
                cs3 = cs[:].rearrange("p (t x) -> p t x", x=128)
                magsl = magb3[:, h * CHT:(h + 1) * CHT, :]
                nc.vector.tensor_tensor(out=cs3[:, :, 0:D],
                                        in0=trig3[:, :, 0:D],
                                        in1=magsl, op=ALU.mult)
                nc.vector.tensor_tensor(out=cs3[:, :, D:128],
                                        in0=trig3[:, :, D:128],
                                        in1=magsl, op=ALU.mult)
                for qd in range(CHT // 4):
                    tp = apsp.tile([128, 512], f16, tag="tp")
                    for k in range(4):
                        i = qd * 4 + k
                        nc.tensor.transpose(out=tp[:, k * 128:(k + 1) * 128],
                                            in_=cs[:, i * 128:(i + 1) * 128],
                                            identity=ident[:])
                    c0 = (h * CHT + qd * 4) * 128
                    nc.vector.tensor_copy(out=xt[:, c0:c0 + 512], in_=tp[:])

            def emit_pv(pvt, p2, g):
                # start=True clears has_written for the WHOLE bank, so only
                # the first matmul touching each pv bank may set it; the
                # sibling 264-offset slice first-writes via cleared bits.
                for j in range(KCG):
                    kc = g * KCG + j
                    for qs in range(4):
                        pv = pvt[qs // 2]
                        o0 = (qs % 2) * 264
                        first = (g == 0 and j == 0 and qs % 2 == 0)
                        last = (g == N_G - 1 and j == KCG - 1 and qs % 2 == 1)
                        nc.tensor.matmul(
                            out=pv[:, o0:o0 + 129],
                            lhsT=p2[:, j * QH + qs * 128:j * QH + (qs + 1) * 128],
                            rhs=mpo[:, kc * MPW:kc * MPW + 129],
                            start=first, stop=last, skip_group_check=True)

            for qh in range(N_QH):
                pva = pvp.tile([128, 512], f32, tag="pva")
                pvb = pvp.tile([128, 512], f32, tag="pvb")
                pvt = (pva, pvb)
                rhs_q = xtq[:, qh * QH:(qh + 1) * QH]
                pend = None
                for g in range(N_G):
                    if qh == 0 and g % (N_G // N_CH) == 0:
                        emit_chunk(g // (N_G // N_CH))
                    # bias inject: j=0 always on PE; j=1 on DVE during qh=1
                    # (DVE has slack there, PE is the bottleneck)
                    dve_j1 = (qh == 1)
                    bias_t = bp.tile([128, GW], f16, tag="bias_t")
                    off = (qh * N_G + g) * slots
                    nc.gpsimd.local_scatter(bias_t[:], esb_m[:, off:off + slots],
                                            eidx_sb[:, off:off + slots],
                                            channels=128, num_elems=GW,
                                            num_idxs=slots)
                    qk = qkp.tile([128, GW], f32, tag="qk")
                    for j in range(KCG):
                        kc = g * KCG + j
                        pe_add = (j == 0) or not dve_j1
                        nc.tensor.matmul(out=qk[:, j * QH:(j + 1) * QH],
                                         lhsT=xt[:, kc * 128:(kc + 1) * 128],
                                         rhs=rhs_q, start=True,
                                         stop=not pe_add)
                        if pe_add:
                            nc.tensor.matmul(out=qk[:, j * QH:(j + 1) * QH],
                                             lhsT=ident[:],
                                             rhs=bias_t[:, j * QH:(j + 1) * QH],
                                             start=False, stop=True)
                    if dve_j1:
                        nc.vector.tensor_tensor(out=qk[:, QH:GW],
                                                in0=qk[:, QH:GW],
                                                in1=bias_t[:, QH:GW],
                                                op=ALU.add)
                    p2 = p2p.tile([128, GW], bf16, tag="p2")
                    nc.scalar.activation(out=p2[:], in_=qk[:], func=AF.Exp)
                    if pend is not None:
                        emit_pv(pvt, pend[0], pend[1])
                    pend = (p2, g)
                emit_pv(pvt, pend[0], pend[1])

                # epilogue: per 128-q chunk, divide by denominator column
                for qs in range(4):
                    pv = pvt[qs // 2]
                    o0 = (qs % 2) * 264
                    rec = obp.tile([128, 1], f32, tag="rec")
                    nc.vector.reciprocal(out=rec[:], in_=pv[:, o0 + 128:o0 + 129])
                    o_t = obp.tile([128, 128], f32, tag="o_t")
                    nc.vector.tensor_scalar(o_t[:], pv[:, o0:o0 + 128], rec[:],
                                            None, ALU.mult)
                    r0 = qh * QH + qs * 128
                    nc.sync.dma_start(out=out_d[r0:r0 + 128, :], in_=o_t[:])

    nc.compile()
    return nc


def _prep_edges(edge_index, edge_attr):
    """Dedup (last wins, matching CPU XLA scatter-set) and bucket edges by
    (core, query-half, kc-pair group, dst%128)."""
    src = np.asarray(edge_index[0], dtype=np.int64)
    dst = np.asarray(edge_index[1], dtype=np.int64)
    keys = src * N + dst
    order = np.argsort(keys, kind="stable")
    ks = keys[order]
    run_last = np.flatnonzero(np.r_[ks[1:] != ks[:-1], True])
    kept = order[run_last]  # stable sort => last occurrence per duplicate key
    s, d = src[kept], dst[kept]
    attr = np.asarray(edge_attr, dtype=np.float32)[kept]

    core = s // NQ
    qh = (s % NQ) // QH
    g = d // (KCG * KC)
    j = (d % (KCG * KC)) // KC
    ch = d % 128
    col = j * QH + (s % QH)

    cell = ((core * N_QH + qh) * N_G + g) * 128 + ch
    o2 = np.argsort(cell, kind="stable")
    cell_s = cell[o2]
    first = np.r_[True, cell_s[1:] != cell_s[:-1]]
    run_id = np.cumsum(first) - 1
    run_start = np.flatnonzero(first)
    slot = np.arange(len(cell_s)) - run_start[run_id]
    slots = int(max(int(slot.max()) + 1 if len(slot) else 1, 4))
    slots = (slots + 1) // 2 * 2  # even

    tot = N_QH * N_G * slots
    eidx = np.full((CORES, 128, tot), -1, dtype=np.int16)
    eattr = np.zeros((CORES, 128, 4, tot), dtype=np.float16)
    cs_, qhs, gs, chs, cols = core[o2], qh[o2], g[o2], ch[o2], col[o2]
    off = (qhs * N_G + gs) * slots + slot
    eidx[cs_, chs, off] = cols.astype(np.int16)
    a2 = attr[o2]
    for k in range(EDGE_DIM):
        eattr[cs_, chs, k, off] = a2[:, k].astype(np.float16)
    return eidx, eattr.reshape(CORES, 128, 4 * tot), slots


def kernel(mag, phase, edge_index, edge_attr, W, b):
    global LAST_RESULTS
    bf16_np = mybir.dt.np(bf16)
    mag = np.ascontiguousarray(np.asarray(mag, dtype=np.float32))
    phase = np.ascontiguousarray(np.asarray(phase, dtype=np.float32))
    W = np.ascontiguousarray(np.asarray(W, dtype=np.float32))
    bvec = np.ascontiguousarray(np.asarray(b, dtype=np.float32).reshape(D, 1))
    mag16 = mag.astype(np.float16)
    mpo_full = np.zeros((N, MPW), dtype=bf16_np)
    mpo_full[:, 0:D] = mag.astype(bf16_np)
    mpo_full[:, D:2 * D] = phase.astype(bf16_np)
    mpo_full[:, 2 * D] = 1.0
    mpo_full = np.ascontiguousarray(mpo_full)

    eidx, eattr, slots = _prep_edges(edge_index, edge_attr)

    if slots not in _cache:
        _cache[slots] = _build(slots)
    nc = _cache[slots]

    in_maps = []
    for c in range(CORES):
        in_maps.append({
            "phase": phase,
            "phq": np.ascontiguousarray(phase[c * NQ:(c + 1) * NQ]),
            "mag": mag16,
            "mq": np.ascontiguousarray(mag16[c * NQ:(c + 1) * NQ]),
            "mpo": mpo_full,
            "eidx": np.ascontiguousarray(eidx[c]),
            "eattr": np.ascontiguousarray(eattr[c]),
            "W": W, "bvec": bvec,
        })
    res = run_bass_kernel_spmd(nc, in_maps, core_ids=list(range(CORES)))
    LAST_RESULTS = res

    new_mag = np.empty((N, D), dtype=np.float32)
    new_phase = np.empty((N, D), dtype=np.float32)
    for c in range(CORES):
        o = np.asarray(res.results[c]["out"], dtype=np.float32)
        new_mag[c * NQ:(c + 1) * NQ] = o[:, 0:D]
        new_phase[c * NQ:(c + 1) * NQ] = o[:, D:2 * D]
    return new_mag, new_phase
